# revision 2
# baseline (speedup 1.0000x reference)
"""Multi-head attention (B=4, L=2048, D=1024, H=16, causal) on 8 trn2 cores.

Sharding: core c handles batch b=c//2 and head-group hg=c%2 (8 heads = 512 of
the 1024 projection dims).  Each core computes Q/K/V projections for its
(batch, head-group), causal attention for its 8 heads, and a partial output
projection (its 512 ctx dims x full Wo rows slice).  The two cores sharing a
batch produce additive partials; the host sums the pair.

vs kernel.py (the S^T-layout baseline): the P@V matmul is FLIPPED —
ctx[lq=128, dk+1=65] = pt_slice[lk=128, lq=128]^T @ vh[lk=128, 65] — so each
streamed PE row does full 128x128 MACs instead of 65x128 (the old layout
wasted half the PE columns on M=65).  PV cost drops from N=512 to N=65 per
(lk-tile, head, lq-subtile).  The softmax denominator (ones column of vh) now
lands PER-PARTITION (one lq row each), so normalization is a DVE per-partition
tensor_scalar instead of the old reciprocal-row + PE-broadcast-matmul dance.
The normalized ctx[lq, dout] is transposed back to ctxT[dout, lq] with PE
is_transpose matmuls (f32, via an identity) for the output projection.

Inputs arrive fp32; the host pre-transposes/casts q/k/v to bf16 [D, L] per
core.  All matmuls run bf16 with fp32 PSUM accumulation.
"""

import os as _os
import numpy as np
import ml_dtypes

import concourse.bass as bass
import concourse.mybir as mybir
from concourse.tile import TileContext
from concourse.masks import make_identity

BF16 = mybir.dt.bfloat16
F32 = mybir.dt.float32

B, L, D, H, DK = 4, 2048, 1024, 16, 64
HPC = 8            # heads per core
DOUT = 512         # projection dims per core
NCHUNK = 4         # lq chunks of 512
KT = 8             # k-tiles over D
SCALE = 1.0 / np.sqrt(DK)

_PROGRAM = None


def _legalize_waits(nc):
    """This walrus build rejects >1 semaphore wait per instruction; split
    extras onto single-wait no-op carriers inserted just before, same engine."""
    for fn in nc.m.functions:
        for blk in fn.blocks:
            insts = blk.instructions
            i = 0
            while i < len(insts):
                inst = insts[i]
                si = inst.sync_info
                waits = list(si.on_wait) if (si and si.on_wait) else []
                if len(waits) > 1:
                    si.on_wait = waits[-1:]
                    carriers = [
                        mybir.InstNoOp(
                            name=nc.get_next_instruction_name(),
                            engine=inst.engine,
                            ins=[],
                            outs=[],
                            sync_info=mybir.SyncInfo(on_wait=[w], on_update=[]),
                        )
                        for w in waits[:-1]
                    ]
                    insts[i:i] = carriers
                    i += len(carriers)
                i += 1


def build_program(reps=1, num_devices=8):
    """reps>1 repeats the whole kernel body inside one NEFF — used only by
    the benchmark to amortize the per-dispatch cost."""
    nc = bass.Bass("TRN2", target_bir_lowering=False, debug=False,
                   num_devices=num_devices)

    qT = nc.declare_dram_parameter("qT", [D, L], BF16, isOutput=False)
    kT = nc.declare_dram_parameter("kT", [D, L], BF16, isOutput=False)
    vT = nc.declare_dram_parameter("vT", [D, L], BF16, isOutput=False)
    wq = nc.declare_dram_parameter("wq", [D, DOUT], BF16, isOutput=False)
    wk = nc.declare_dram_parameter("wk", [D, DOUT], BF16, isOutput=False)
    wv = nc.declare_dram_parameter("wv", [D, DOUT], BF16, isOutput=False)
    wo = nc.declare_dram_parameter("wo", [DOUT, D], BF16, isOutput=False)
    bq_col = nc.declare_dram_parameter("bq_col", [DOUT, 1], F32, isOutput=False)
    bvbc = nc.declare_dram_parameter("bvbc", [128, DOUT], BF16, isOutput=False)
    bobc = nc.declare_dram_parameter("bobc", [128, D], F32, isOutput=False)
    maskB = nc.declare_dram_parameter("maskB", [128, 1024], BF16, isOutput=False)
    out = nc.declare_dram_parameter("out", [L, D], F32, isOutput=True)

    with TileContext(nc) as tc:
        with (
            tc.tile_pool(name="wpool", bufs=1) as wpool,
            tc.tile_pool(name="vtpool", bufs=1) as vtpool,
            tc.tile_pool(name="big", bufs=1) as big,
            tc.tile_pool(name="qin", bufs=int(_os.environ.get("K_QIN", "6"))) as qin,
            tc.tile_pool(name="kin", bufs=int(_os.environ.get("K_KIN", "6"))) as kin,
            tc.tile_pool(name="ppool", bufs=int(_os.environ.get("K_PPOOL", "4"))) as ppool,
            tc.tile_pool(name="cscp", bufs=3) as cscp,
            tc.tile_pool(name="rcp", bufs=3) as rcp,
            tc.tile_pool(name="outsb", bufs=int(_os.environ.get("K_OUTSB", "4"))) as outsb,
            tc.tile_pool(name="mmps", bufs=2, space="PSUM") as mmps,
            tc.tile_pool(name="sps", bufs=2, space="PSUM") as spsp,
            tc.tile_pool(name="ctxps", bufs=1, space="PSUM") as ctxps,
        ):
            # ---- constants / weights ----
            wq_sb = [wpool.tile([128, DOUT], BF16, name=f"wq{k}", tag=f"wq{k}") for k in range(KT)]
            wkw = wpool.tile([128, KT * DOUT], BF16, name="wkw", tag="wkw")
            wvw = wpool.tile([128, KT * DOUT], BF16, name="wvw", tag="wvw")
            wow = wpool.tile([128, 4 * D], BF16, name="wow", tag="wow")
            wk_sb = [wkw[:, k * DOUT:(k + 1) * DOUT] for k in range(KT)]
            wv_sb = [wvw[:, k * DOUT:(k + 1) * DOUT] for k in range(KT)]
            wo_sb = [wow[:, k * D:(k + 1) * D] for k in range(4)]
            bvbc_sb = wpool.tile([128, DOUT], BF16, name="bvbc_sb", tag="bvbc_sb")
            bobc_sb = wpool.tile([128, D], F32, name="bobc_sb", tag="bobc_sb")
            for k in range(KT):
                nc.sync.dma_start(out=wq_sb[k][:], in_=wq[k * 128:(k + 1) * 128, :])
            mask_sb = wpool.tile([128, 1024], BF16, name="mask_sb", tag="mask_sb")
            bq4 = wpool.tile([128, 4], F32, name="bq4", tag="bq4")
            nc.sync.dma_start(
                out=bq4[:], in_=bq_col.rearrange("(m p) one -> p (m one)", p=128))
            bq_sb = [bq4[:, m:m + 1] for m in range(4)]
            # f32 identity for the PE ctx transposes
            identF = wpool.tile([128, 128], F32, name="identF", tag="identF")
            make_identity(nc, identF[:])

            def late_const_loads():
                # issued after chunk 0's q tiles so the big mask tile doesn't
                # delay the first projection matmuls
                nc.sync.dma_start(out=mask_sb[:], in_=maskB[:])

            bigvT = vtpool.tile([128, KT * L], BF16, name="bigvT", tag="bigvT")
            vT_sb = [bigvT[:, k * L:(k + 1) * L] for k in range(KT)]

            # ---- persistent activations ----
            qhT_sb = [big.tile([128, L], BF16, name=f"qhT{m}", tag=f"qhT{m}") for m in range(4)]
            khT_sb = [big.tile([128, L], BF16, name=f"khT{m}", tag=f"khT{m}") for m in range(4)]
            ctxT_sb = [big.tile([128, L], BF16, name=f"ctxT{m}", tag=f"ctxT{m}") for m in range(4)]
            vh_sb = [big.tile([128, HPC * 65], BF16, name=f"vh{t}", tag=f"vh{t}") for t in range(16)]
            for t in range(16):
                # ones denominator columns, written once (v_proj only
                # overwrites the value columns)
                nc.vector.memset(
                    vh_sb[t].rearrange("p (h c) -> p h c", c=65)[:, :, 64:65], 1.0)

            def qk_proj_chunk(n, src, w_sb, dst, pool, pfx, bias):
                tiles = []
                s3 = src.rearrange("(k p) c -> p k c", p=128)
                for kp in range(KT // 2):
                    t = pool.tile([128, 1024], BF16, name=f"{pfx}in", tag=f"{pfx}in")
                    nc.sync.dma_start(
                        out=t.rearrange("p (k c) -> p k c", c=512),
                        in_=s3[:, 2 * kp:2 * kp + 2, n * 512:(n + 1) * 512])
                    tiles.append(t[:, 0:512])
                    tiles.append(t[:, 512:1024])
                for m in range(4):
                    ps = mmps.tile([128, 512], F32, name="mmtile", tag="mmtile")
                    for k in range(KT):
                        nc.tensor.matmul(
                            ps[:], w_sb[k][:, m * 128:(m + 1) * 128], tiles[k][:],
                            start=(k == 0), stop=(k == KT - 1))
                    dsts = dst[m][:, n * 512:(n + 1) * 512]
                    if bias is not None:
                        nc.scalar.activation(
                            out=dsts, in_=ps[:],
                            func=mybir.ActivationFunctionType.Identity,
                            bias=bias[m][:])
                    else:
                        nc.scalar.activation(
                            out=dsts, in_=ps[:],
                            func=mybir.ActivationFunctionType.Copy)

            def v_proj_tile(mt):
                ps = mmps.tile([128, 512], F32, name="mmtile", tag="mmtile")
                for k in range(KT):
                    nc.tensor.matmul(
                        ps[:], vT_sb[k][:, mt * 128:(mt + 1) * 128], wv_sb[k][:],
                        start=(k == 0), stop=(k == KT - 1))
                dst3 = vh_sb[mt].rearrange("p (h c) -> p h c", c=65)
                nc.vector.tensor_add(
                    dst3[:, :, 0:64], ps.rearrange("p (h c) -> p h c", c=64),
                    bvbc_sb.rearrange("p (h c) -> p h c", c=64))

            def out_proj_piece(jj, mt2):
                row0 = jj * 512 + mt2 * 128
                ob = outsb.tile([128, 1024], F32, name="ob", tag="ob")
                for n2 in range(2):
                    ps = mmps.tile([128, 512], F32, name="mmtile", tag="mmtile")
                    for ktile in range(4):
                        nc.tensor.matmul(
                            ps[:],
                            ctxT_sb[ktile][:, row0:row0 + 128],
                            wo_sb[ktile][:, n2 * 512:(n2 + 1) * 512],
                            start=(ktile == 0), stop=(ktile == 3))
                    nc.vector.tensor_add(
                        ob[:, n2 * 512:(n2 + 1) * 512], ps[:],
                        bobc_sb[:, n2 * 512:(n2 + 1) * 512])
                nc.sync.dma_start(out=out[row0:row0 + 128, :], in_=ob[:])

            # Deferred transpose tail: the ctx transposes must not enter the
            # in-order PE queue until their ctn operand (DVE normalize) is
            # surely ready, or every later PE instruction stalls behind them.
            # The tail is emitted a few iterations into the NEXT i-loop (or
            # before the final out-proj).
            pending_norm = [None]

            def flush_norm():
                if pending_norm[0] is not None:
                    pending_norm[0]()
                    pending_norm[0] = None

            def attention_chunk(j):
                ilast = 4 * (j + 1) - 1
                for hp in range(4):
                    ctxs = (
                        ctxps.tile([128, 4 * 65], F32, name="ctxA", tag="ctxA"),
                        ctxps.tile([128, 4 * 65], F32, name="ctxB", tag="ctxB"),
                    )
                    flush_i = min(int(_os.environ.get("K_FLUSH", "5")), ilast)
                    for i in range(ilast + 1):
                        if i == flush_i:
                            flush_norm()
                        # columns [d, 512) of the lq chunk are reachable from
                        # lk-tile i under the causal mask; d=0 for full blocks
                        d = max(0, (i - 4 * j) * 128)
                        straddle = i >= 4 * j
                        sp = spsp.tile([128, 1024], F32, name="sps", tag="sps")
                        for h2, po in enumerate((0, 64)):
                            nc.tensor.matmul(
                                sp[:, h2 * 512 + d:(h2 + 1) * 512],
                                khT_sb[hp][po:po + 64, i * 128:(i + 1) * 128],
                                qhT_sb[hp][po:po + 64, j * 512 + d:(j + 1) * 512],
                                start=True, stop=True, tile_position=(po, 0))
                        pt = ppool.tile([128, 1024], BF16, name="pt", tag="pt")
                        if d == 0:
                            nc.scalar.activation(
                                out=pt[:], in_=sp[:],
                                func=mybir.ActivationFunctionType.Exp, scale=float(SCALE))
                        else:
                            sp3 = sp.rearrange("p (g c) -> p g c", g=2)
                            pt3 = pt.rearrange("p (g c) -> p g c", g=2)
                            nc.scalar.activation(
                                out=pt3[:, :, d:512], in_=sp3[:, :, d:512],
                                func=mybir.ActivationFunctionType.Exp, scale=float(SCALE))
                        if straddle:
                            # diagonal 128-col strip: causal mask multiply on
                            # the otherwise-idle GPSIMD/Pool engine
                            nc.gpsimd.tensor_mul(
                                pt[:, d:d + 128], pt[:, d:d + 128],
                                mask_sb[:, 512:640])
                            nc.gpsimd.tensor_mul(
                                pt[:, 512 + d:512 + d + 128],
                                pt[:, 512 + d:512 + d + 128],
                                mask_sb[:, 512:640])
                        smin = max(0, i - 4 * j)
                        for h2 in range(2):
                            h = hp * 2 + h2
                            for s in range(smin, 4):
                                # PSUM start clears has_written for the WHOLE
                                # bank; later start=False writes overwrite
                                # where clear / accumulate where set — so one
                                # start (first matmul into the tile) serves
                                # all four subtile groups
                                nc.tensor.matmul(
                                    ctxs[h2][:, 65 * s:65 * s + 65],
                                    pt[:, h2 * 512 + s * 128:h2 * 512 + (s + 1) * 128],
                                    vh_sb[i][:, h * 65:(h + 1) * 65],
                                    start=(i == 0 and s == 0),
                                    stop=(i == ilast and s == 3))
                    # inline DVE tail: per-partition denominators -> recip ->
                    # normalize+drain ctx into SBUF (f32, for the transpose)
                    rc = rcp.tile([128, 8], F32, name="rc", tag="rc")
                    rc3 = rc.rearrange("p (s one) -> p s one", one=1)
                    ctn = cscp.tile([128, 512], F32, name="ctn", tag="ctn")
                    for h2 in range(2):
                        ctx3 = ctxs[h2].rearrange("p (s c) -> p s c", c=65)
                        nc.vector.reciprocal(
                            out=rc3[:, 4 * h2:4 * h2 + 4, :],
                            in_=ctx3[:, :, 64:65])
                        for s in range(4):
                            nc.vector.tensor_scalar_mul(
                                ctn[:, s * 128 + h2 * 64:s * 128 + h2 * 64 + 64],
                                ctxs[h2][:, 65 * s:65 * s + 64],
                                rc[:, 4 * h2 + s:4 * h2 + s + 1])

                    def norm_tail(hp=hp, j=j, ctn=ctn):
                        # sps pool, NOT mmps: at final-flush time both mmps
                        # buffers are held by the staged ps0 matmuls whose
                        # readers are emitted after this flush (deadlock)
                        tp = spsp.tile([128, 1024], F32, name="sps", tag="sps")[:, 0:512]
                        for s in range(4):
                            nc.tensor.matmul(
                                tp[:, s * 128:(s + 1) * 128],
                                ctn[:, s * 128:(s + 1) * 128],
                                identF[:], is_transpose=True,
                                start=(s == 0), stop=(s == 3))
                        nc.vector.tensor_copy(
                            ctxT_sb[hp][:, j * 512:(j + 1) * 512], tp[:])

                    pending_norm[0] = norm_tail
                    if j >= 1:
                        out_proj_piece(j - 1, hp)

            def kernel_body(first):
                for n in range(NCHUNK):
                    qk_proj_chunk(n, qT, wq_sb, qhT_sb, qin, "q", bq_sb)
                    if n == 0 and first:
                        late_const_loads()
                        for kp in range(KT // 2):
                            nc.sync.dma_start(
                                out=wkw.rearrange("p (k c) -> p k c", c=DOUT)[:, 2 * kp:2 * kp + 2, :],
                                in_=wk.rearrange("(k p) c -> p k c", p=128)[:, 2 * kp:2 * kp + 2, :])
                    qk_proj_chunk(n, kT, wk_sb, khT_sb, kin, "k", None)
                    if n == 0 and first:
                        for kp in range(KT // 2):
                            nc.sync.dma_start(
                                out=wvw.rearrange("p (k c) -> p k c", c=DOUT)[:, 2 * kp:2 * kp + 2, :],
                                in_=wv.rearrange("(k p) c -> p k c", p=128)[:, 2 * kp:2 * kp + 2, :])
                        nc.sync.dma_start(out=bvbc_sb[:], in_=bvbc[:])
                    # stream this chunk's vT column slice
                    if first:
                        bv3 = bigvT.rearrange("p (k c) -> p k c", c=L)
                        vs3 = vT.rearrange("(k p) c -> p k c", p=128)
                        for kp in range(KT // 2):
                            nc.sync.dma_start(
                                out=bv3[:, 2 * kp:2 * kp + 2, n * 512:(n + 1) * 512],
                                in_=vs3[:, 2 * kp:2 * kp + 2, n * 512:(n + 1) * 512])
                    for mt in range(4 * n, 4 * n + 4):
                        v_proj_tile(mt)
                    if n == 0 and first:
                        for kp in range(2):
                            nc.sync.dma_start(
                                out=wow.rearrange("p (k c) -> p k c", c=D)[:, 2 * kp:2 * kp + 2, :],
                                in_=wo.rearrange("(k p) c -> p k c", p=128)[:, 2 * kp:2 * kp + 2, :])
                        nc.sync.dma_start(out=bobc_sb[:], in_=bobc[:])
                    attention_chunk(n)
                # tail overlap: the first final piece's ktile 0..2 matmuls only
                # need the already-transposed head-pairs, so they run while the
                # last head-pair's deferred transpose chain completes
                row0 = (NCHUNK - 1) * 512
                ob0 = outsb.tile([128, 1024], F32, name="ob", tag="ob")
                ps0 = []
                for n2 in range(2):
                    ps = mmps.tile([128, 512], F32, name="mmtile", tag="mmtile")
                    for ktile in range(3):
                        nc.tensor.matmul(
                            ps[:],
                            ctxT_sb[ktile][:, row0:row0 + 128],
                            wo_sb[ktile][:, n2 * 512:(n2 + 1) * 512],
                            start=(ktile == 0), stop=False)
                    ps0.append(ps)
                flush_norm()
                for n2 in range(2):
                    nc.tensor.matmul(
                        ps0[n2][:],
                        ctxT_sb[3][:, row0:row0 + 128],
                        wo_sb[3][:, n2 * 512:(n2 + 1) * 512],
                        start=False, stop=True)
                    nc.vector.tensor_add(
                        ob0[:, n2 * 512:(n2 + 1) * 512], ps0[n2][:],
                        bobc_sb[:, n2 * 512:(n2 + 1) * 512])
                nc.sync.dma_start(out=out[row0:row0 + 128, :], in_=ob0[:])
                for mt2 in range(1, 3):
                    out_proj_piece(NCHUNK - 1, mt2)
                # last piece: per-half DMAs so the first half's writeback
                # overlaps the second half's bias add, shortening the drain
                rowL = (NCHUNK - 1) * 512 + 3 * 128
                obL = outsb.tile([128, 1024], F32, name="ob", tag="ob")
                for n2 in range(2):
                    ps = mmps.tile([128, 512], F32, name="mmtile", tag="mmtile")
                    for ktile in range(4):
                        nc.tensor.matmul(
                            ps[:],
                            ctxT_sb[ktile][:, rowL:rowL + 128],
                            wo_sb[ktile][:, n2 * 512:(n2 + 1) * 512],
                            start=(ktile == 0), stop=(ktile == 3))
                    nc.vector.tensor_add(
                        obL[:, n2 * 512:(n2 + 1) * 512], ps[:],
                        bobc_sb[:, n2 * 512:(n2 + 1) * 512])
                    nc.sync.dma_start(
                        out=out[rowL:rowL + 128, n2 * 512:(n2 + 1) * 512],
                        in_=obL[:, n2 * 512:(n2 + 1) * 512])

            kernel_body(first=True)
            for _ in range(reps - 1):
                kernel_body(first=False)

    _legalize_waits(nc)
    return nc


def get_program():
    global _PROGRAM
    if _PROGRAM is None:
        _PROGRAM = build_program()
    return _PROGRAM


def make_in_maps(q, k, v, Wq, bq, Wk, bk, Wv, bv, Wo, bo):
    bf = ml_dtypes.bfloat16
    q = np.asarray(q, np.float32); k = np.asarray(k, np.float32)
    v = np.asarray(v, np.float32)
    Wq = np.asarray(Wq, np.float32); Wk = np.asarray(Wk, np.float32)
    Wv = np.asarray(Wv, np.float32); Wo = np.asarray(Wo, np.float32)
    bq = np.asarray(bq, np.float32); bv = np.asarray(bv, np.float32)
    bo = np.asarray(bo, np.float32)

    # causal sliding mask table: B[p, c] = 1.0 iff c >= p + 512
    p = np.arange(128)[:, None]
    c = np.arange(1024)[None, :]
    maskB = (c >= p + 512).astype(bf)

    qTb = [np.ascontiguousarray(q[b].T.astype(bf)) for b in range(B)]
    kTb = [np.ascontiguousarray(k[b].T.astype(bf)) for b in range(B)]
    vTb = [np.ascontiguousarray(v[b].T.astype(bf)) for b in range(B)]

    in_maps = []
    for core in range(8):
        b, hg = core // 2, core % 2
        hs = hg * DOUT
        in_maps.append({
            "qT": qTb[b], "kT": kTb[b], "vT": vTb[b],
            "wq": np.ascontiguousarray(Wq[:, hs:hs + DOUT].astype(bf)),
            "wk": np.ascontiguousarray(Wk[:, hs:hs + DOUT].astype(bf)),
            "wv": np.ascontiguousarray(Wv[:, hs:hs + DOUT].astype(bf)),
            "wo": np.ascontiguousarray(Wo[hs:hs + DOUT, :].astype(bf)),
            "bq_col": np.ascontiguousarray(bq[hs:hs + DOUT].reshape(DOUT, 1)),
            "bvbc": np.ascontiguousarray(
                np.tile(bv[hs:hs + DOUT].reshape(1, DOUT), (128, 1)).astype(bf)),
            "bobc": np.ascontiguousarray(
                np.tile((bo / 2.0).reshape(1, D), (128, 1)).astype(np.float32)),
            "maskB": maskB,
        })
    return in_maps


def assemble_output(results):
    out = np.empty((B, L, D), np.float32)
    for b in range(B):
        out[b] = results[2 * b]["out"] + results[2 * b + 1]["out"]
    return out


def kernel(q, k, v, attn_mask, Wq, bq, Wk, bk, Wv, bv, Wo, bo):
    from concourse.bass_utils import run_bass_kernel_spmd

    nc = get_program()
    in_maps = make_in_maps(q, k, v, Wq, bq, Wk, bk, Wv, bv, Wo, bo)
    last_err = None
    for _ in range(3):  # retry transient device errors (NRT_EXEC_UNIT_...)
        try:
            res = run_bass_kernel_spmd(nc, in_maps, list(range(8)), trace=False)
            return assemble_output(res.results)
        except Exception as e:  # noqa: BLE001
            last_err = e
    raise last_err


# revision 3
# speedup vs baseline: 1.1261x; 1.1261x over previous
"""Multi-head attention (B=4, L=2048, D=1024, H=16, causal) on 8 trn2 cores.

Sharding: core c handles batch b=c//2 and head-group hg=c%2 (8 heads = 512 of
the 1024 projection dims).  Each core computes Q/K/V projections for its
(batch, head-group), causal attention for its 8 heads, and a partial output
projection; the two cores sharing a batch produce additive partials the host
sums.

vs kernel2 (flipped-PV): the attention i-loop is ACT-bound (exp) while the
projection phases are PE-bound, so kernel3 dissolves the phase boundary: all
non-attention PE work (next chunk's Q/K/V projection matmuls, prior chunk's
output-projection pieces, the deferred ctx transposes) flows through a global
FIFO work queue drained a few units after each P@V group inside the attention
i-loop — PE fills its exp-wait gaps with projection work.  Scores are emitted
one i ahead (sp double buffer) so exp(i) overlaps scores(i+1) instead of
stalling PE.  Projection drains move from ACT to DVE (ACT must stay exp-only
once projections overlap attention), and the ctx transposes run bf16 (half
the PE rows of f32) into a bitcast view of an mmps PSUM tile.

Inputs arrive fp32; the host pre-transposes/casts q/k/v to bf16 [D, L] per
core.  All matmuls run bf16 with fp32 PSUM accumulation.
"""

import os as _os
from collections import deque
import numpy as np
import ml_dtypes

import concourse.bass as bass
import concourse.mybir as mybir
from concourse.tile import TileContext
from concourse.masks import make_identity

BF16 = mybir.dt.bfloat16
F32 = mybir.dt.float32

B, L, D, H, DK = 4, 2048, 1024, 16, 64
HPC = 8            # heads per core
DOUT = 512         # projection dims per core
NCHUNK = 4         # lq chunks of 512
KT = 8             # k-tiles over D
SCALE = 1.0 / np.sqrt(DK)

_PROGRAM = None


def _legalize_waits(nc):
    """This walrus build rejects >1 semaphore wait per instruction; split
    extras onto single-wait no-op carriers inserted just before, same engine."""
    for fn in nc.m.functions:
        for blk in fn.blocks:
            insts = blk.instructions
            i = 0
            while i < len(insts):
                inst = insts[i]
                si = inst.sync_info
                waits = list(si.on_wait) if (si and si.on_wait) else []
                if len(waits) > 1:
                    si.on_wait = waits[-1:]
                    carriers = [
                        mybir.InstNoOp(
                            name=nc.get_next_instruction_name(),
                            engine=inst.engine,
                            ins=[],
                            outs=[],
                            sync_info=mybir.SyncInfo(on_wait=[w], on_update=[]),
                        )
                        for w in waits[:-1]
                    ]
                    insts[i:i] = carriers
                    i += len(carriers)
                i += 1


def build_program(reps=1, num_devices=8):
    """reps>1 repeats the whole kernel body inside one NEFF — used only by
    the benchmark to amortize the per-dispatch cost."""
    nc = bass.Bass("TRN2", target_bir_lowering=False, debug=False,
                   num_devices=num_devices)

    qT = nc.declare_dram_parameter("qT", [D, L], BF16, isOutput=False)
    kT = nc.declare_dram_parameter("kT", [D, L], BF16, isOutput=False)
    vT = nc.declare_dram_parameter("vT", [D, L], BF16, isOutput=False)
    wq = nc.declare_dram_parameter("wq", [D, DOUT], BF16, isOutput=False)
    wk = nc.declare_dram_parameter("wk", [D, DOUT], BF16, isOutput=False)
    wv = nc.declare_dram_parameter("wv", [D, DOUT], BF16, isOutput=False)
    wo = nc.declare_dram_parameter("wo", [DOUT, D], BF16, isOutput=False)
    bq_col = nc.declare_dram_parameter("bq_col", [DOUT, 1], F32, isOutput=False)
    bvbc = nc.declare_dram_parameter("bvbc", [128, DOUT], BF16, isOutput=False)
    bobc = nc.declare_dram_parameter("bobc", [128, D], F32, isOutput=False)
    maskB = nc.declare_dram_parameter("maskB", [128, 1024], BF16, isOutput=False)
    out = nc.declare_dram_parameter("out", [L, D], F32, isOutput=True)

    with TileContext(nc) as tc:
        with (
            tc.tile_pool(name="wpool", bufs=1) as wpool,
            tc.tile_pool(name="vtpool", bufs=1) as vtpool,
            tc.tile_pool(name="big", bufs=1) as big,
            tc.tile_pool(name="qin", bufs=int(_os.environ.get("K_QIN", "6"))) as qin,
            tc.tile_pool(name="kin", bufs=int(_os.environ.get("K_KIN", "6"))) as kin,
            tc.tile_pool(name="ppool", bufs=int(_os.environ.get("K_PPOOL", "4"))) as ppool,
            tc.tile_pool(name="cscp", bufs=3) as cscp,
            tc.tile_pool(name="rcp", bufs=3) as rcp,
            tc.tile_pool(name="outsb", bufs=int(_os.environ.get("K_OUTSB", "4"))) as outsb,
            tc.tile_pool(name="mmps", bufs=2, space="PSUM") as mmps,
            tc.tile_pool(name="sps", bufs=2, space="PSUM") as spsp,
            tc.tile_pool(name="ctxps", bufs=1, space="PSUM") as ctxps,
        ):
            # ---- constants / weights ----
            wq_sb = [wpool.tile([128, DOUT], BF16, name=f"wq{k}", tag=f"wq{k}") for k in range(KT)]
            wkw = wpool.tile([128, KT * DOUT], BF16, name="wkw", tag="wkw")
            wvw = wpool.tile([128, KT * DOUT], BF16, name="wvw", tag="wvw")
            wow = wpool.tile([128, 4 * D], BF16, name="wow", tag="wow")
            wk_sb = [wkw[:, k * DOUT:(k + 1) * DOUT] for k in range(KT)]
            wv_sb = [wvw[:, k * DOUT:(k + 1) * DOUT] for k in range(KT)]
            wo_sb = [wow[:, k * D:(k + 1) * D] for k in range(4)]
            bvbc_sb = wpool.tile([128, DOUT], BF16, name="bvbc_sb", tag="bvbc_sb")
            bobc_sb = wpool.tile([128, D], F32, name="bobc_sb", tag="bobc_sb")
            for k in range(KT):
                nc.sync.dma_start(out=wq_sb[k][:], in_=wq[k * 128:(k + 1) * 128, :])
            mask_sb = wpool.tile([128, 1024], BF16, name="mask_sb", tag="mask_sb")
            bq4 = wpool.tile([128, 4], F32, name="bq4", tag="bq4")
            nc.sync.dma_start(
                out=bq4[:], in_=bq_col.rearrange("(m p) one -> p (m one)", p=128))
            bq_sb = [bq4[:, m:m + 1] for m in range(4)]
            # bf16 identity for the PE ctx transposes
            identB = wpool.tile([128, 128], BF16, name="identB", tag="identB")
            make_identity(nc, identB[:])

            def late_const_loads():
                nc.sync.dma_start(out=mask_sb[:], in_=maskB[:])

            bigvT = vtpool.tile([128, KT * L], BF16, name="bigvT", tag="bigvT")
            vT_sb = [bigvT[:, k * L:(k + 1) * L] for k in range(KT)]

            # ---- persistent activations ----
            qhT_sb = [big.tile([128, L], BF16, name=f"qhT{m}", tag=f"qhT{m}") for m in range(4)]
            khT_sb = [big.tile([128, L], BF16, name=f"khT{m}", tag=f"khT{m}") for m in range(4)]
            ctxT_sb = [big.tile([128, L], BF16, name=f"ctxT{m}", tag=f"ctxT{m}") for m in range(4)]
            vh_sb = [big.tile([128, HPC * 65], BF16, name=f"vh{t}", tag=f"vh{t}") for t in range(16)]
            for t in range(16):
                nc.vector.memset(
                    vh_sb[t].rearrange("p (h c) -> p h c", c=65)[:, :, 64:65], 1.0)

            # ---- global PE filler work queue ----
            # closures that emit one small PE unit (plus its drains); drained
            # K_DRAIN at a time after each P@V group inside the attention
            # i-loop, so PE fills its exp-wait gaps with projection work
            workq = deque()
            DRAIN = int(_os.environ.get("K_DRAIN", "2"))

            def drain(k=DRAIN):
                for _ in range(min(k, len(workq))):
                    workq.popleft()()

            def flush_all():
                while workq:
                    workq.popleft()()

            def queue_qk_proj(n, src, w_sb, dst, pool, pfx, bias):
                state = {}

                def dmas():
                    tiles = []
                    s3 = src.rearrange("(k p) c -> p k c", p=128)
                    for kp in range(KT // 2):
                        t = pool.tile([128, 1024], BF16, name=f"{pfx}in", tag=f"{pfx}in")
                        nc.sync.dma_start(
                            out=t.rearrange("p (k c) -> p k c", c=512),
                            in_=s3[:, 2 * kp:2 * kp + 2, n * 512:(n + 1) * 512])
                        tiles.append(t[:, 0:512])
                        tiles.append(t[:, 512:1024])
                    state["tiles"] = tiles
                workq.append(dmas)

                for m in range(4):
                    def mk(m=m, k=0):
                        ps = mmps.tile([128, 512], F32, name="mmtile", tag="mmtile")
                        state[f"ps{m}"] = ps
                        nc.tensor.matmul(
                            ps[:], w_sb[0][:, m * 128:(m + 1) * 128],
                            state["tiles"][0][:], start=True, stop=False)
                    workq.append(mk)
                    for k in range(1, KT, 2):
                        def mk2(m=m, k=k):
                            ps = state[f"ps{m}"]
                            for kk in (k, k + 1):
                                if kk < KT:
                                    nc.tensor.matmul(
                                        ps[:], w_sb[kk][:, m * 128:(m + 1) * 128],
                                        state["tiles"][kk][:],
                                        start=False, stop=(kk == KT - 1))
                        workq.append(mk2)

                    def mdrain(m=m):
                        ps = state[f"ps{m}"]
                        dsts = dst[m][:, n * 512:(n + 1) * 512]
                        # DVE, not ACT: these drains now overlap the previous
                        # chunk's attention and ACT must stay exp-only
                        if bias is not None:
                            nc.vector.tensor_scalar_add(dsts, ps[:], bias[m][:])
                        else:
                            nc.vector.tensor_copy(dsts, ps[:])
                    workq.append(mdrain)

            def queue_v_proj(n, first):
                def vdma():
                    if not first:
                        return
                    bv3 = bigvT.rearrange("p (k c) -> p k c", c=L)
                    vs3 = vT.rearrange("(k p) c -> p k c", p=128)
                    for kp in range(KT // 2):
                        nc.sync.dma_start(
                            out=bv3[:, 2 * kp:2 * kp + 2, n * 512:(n + 1) * 512],
                            in_=vs3[:, 2 * kp:2 * kp + 2, n * 512:(n + 1) * 512])
                workq.append(vdma)
                state = {}
                for mt in range(4 * n, 4 * n + 4):
                    def vk(mt=mt, k=0):
                        ps = mmps.tile([128, 512], F32, name="mmtile", tag="mmtile")
                        state[f"ps{mt}"] = ps
                        nc.tensor.matmul(
                            ps[:], vT_sb[0][:, mt * 128:(mt + 1) * 128], wv_sb[0][:],
                            start=True, stop=False)
                    workq.append(vk)
                    for k in range(1, KT, 2):
                        def vk2(mt=mt, k=k):
                            ps = state[f"ps{mt}"]
                            for kk in (k, k + 1):
                                if kk < KT:
                                    nc.tensor.matmul(
                                        ps[:], vT_sb[kk][:, mt * 128:(mt + 1) * 128],
                                        wv_sb[kk][:], start=False,
                                        stop=(kk == KT - 1))
                        workq.append(vk2)

                    def vdrain(mt=mt):
                        ps = state[f"ps{mt}"]
                        dst3 = vh_sb[mt].rearrange("p (h c) -> p h c", c=65)
                        nc.vector.tensor_add(
                            dst3[:, :, 0:64], ps.rearrange("p (h c) -> p h c", c=64),
                            bvbc_sb.rearrange("p (h c) -> p h c", c=64))
                    workq.append(vdrain)

            def queue_out_proj(jj, mt2):
                state = {}

                def half(n2):
                    ps = mmps.tile([128, 512], F32, name="mmtile", tag="mmtile")
                    row0 = jj * 512 + mt2 * 128
                    for ktile in range(4):
                        nc.tensor.matmul(
                            ps[:],
                            ctxT_sb[ktile][:, row0:row0 + 128],
                            wo_sb[ktile][:, n2 * 512:(n2 + 1) * 512],
                            start=(ktile == 0), stop=(ktile == 3))
                    if n2 == 0:
                        state["ob"] = outsb.tile([128, 1024], F32, name="ob", tag="ob")
                    ob = state["ob"]
                    nc.vector.tensor_add(
                        ob[:, n2 * 512:(n2 + 1) * 512], ps[:],
                        bobc_sb[:, n2 * 512:(n2 + 1) * 512])
                    if n2 == 1:
                        nc.sync.dma_start(
                            out=out[row0:row0 + 128, :], in_=ob[:])
                workq.append(lambda: half(0))
                workq.append(lambda: half(1))

            def attention_chunk(j):
                ilast = 4 * (j + 1) - 1

                def scores(i):
                    d = max(0, (i - 4 * j) * 128)
                    sp = spsp.tile([128, 1024], F32, name="sps", tag="sps")
                    for h2, po in enumerate((0, 64)):
                        nc.tensor.matmul(
                            sp[:, h2 * 512 + d:(h2 + 1) * 512],
                            khT_sb[hp][po:po + 64, i * 128:(i + 1) * 128],
                            qhT_sb[hp][po:po + 64, j * 512 + d:(j + 1) * 512],
                            start=True, stop=True, tile_position=(po, 0))
                    return sp

                for hp in range(4):
                    ctxs = (
                        ctxps.tile([128, 4 * 65], F32, name="ctxA", tag="ctxA"),
                        ctxps.tile([128, 4 * 65], F32, name="ctxB", tag="ctxB"),
                    )
                    sp = scores(0)
                    for i in range(ilast + 1):
                        d = max(0, (i - 4 * j) * 128)
                        straddle = i >= 4 * j
                        pt = ppool.tile([128, 1024], BF16, name="pt", tag="pt")
                        if d == 0:
                            nc.scalar.activation(
                                out=pt[:], in_=sp[:],
                                func=mybir.ActivationFunctionType.Exp, scale=float(SCALE))
                        else:
                            sp3 = sp.rearrange("p (g c) -> p g c", g=2)
                            pt3 = pt.rearrange("p (g c) -> p g c", g=2)
                            nc.scalar.activation(
                                out=pt3[:, :, d:512], in_=sp3[:, :, d:512],
                                func=mybir.ActivationFunctionType.Exp, scale=float(SCALE))
                        if i < ilast:
                            sp = scores(i + 1)  # one ahead: overlaps exp(i)
                        if straddle:
                            nc.gpsimd.tensor_mul(
                                pt[:, d:d + 128], pt[:, d:d + 128],
                                mask_sb[:, 512:640])
                            nc.gpsimd.tensor_mul(
                                pt[:, 512 + d:512 + d + 128],
                                pt[:, 512 + d:512 + d + 128],
                                mask_sb[:, 512:640])
                        smin = max(0, i - 4 * j)
                        for h2 in range(2):
                            h = hp * 2 + h2
                            for s in range(smin, 4):
                                # one PSUM start per ctx bank: start clears
                                # has_written for the whole bank; later
                                # start=False writes overwrite where clear,
                                # accumulate where set
                                nc.tensor.matmul(
                                    ctxs[h2][:, 65 * s:65 * s + 65],
                                    pt[:, h2 * 512 + s * 128:h2 * 512 + (s + 1) * 128],
                                    vh_sb[i][:, h * 65:(h + 1) * 65],
                                    start=(i == 0 and s == 0),
                                    stop=(i == ilast and s == 3))
                        drain()
                    # inline DVE tail: per-partition denominators -> recip ->
                    # normalize+drain ctx into SBUF bf16 (for the transpose)
                    rc = rcp.tile([128, 8], F32, name="rc", tag="rc")
                    rc3 = rc.rearrange("p (s one) -> p s one", one=1)
                    ctn = cscp.tile([128, 512], BF16, name="ctn", tag="ctn")
                    for h2 in range(2):
                        ctx3 = ctxs[h2].rearrange("p (s c) -> p s c", c=65)
                        nc.vector.reciprocal(
                            out=rc3[:, 4 * h2:4 * h2 + 4, :],
                            in_=ctx3[:, :, 64:65])
                        for s in range(4):
                            nc.vector.tensor_scalar_mul(
                                ctn[:, s * 128 + h2 * 64:s * 128 + h2 * 64 + 64],
                                ctxs[h2][:, 65 * s:65 * s + 64],
                                rc[:, 4 * h2 + s:4 * h2 + s + 1])

                    def norm_tail(hp=hp, j=j, ctn=ctn):
                        # bf16 transposes into a bitcast view of an mmps tile
                        tpf = mmps.tile([128, 512], F32, name="mmtile", tag="mmtile")
                        tp = tpf.bitcast(BF16)[:, 0:512]
                        for s in range(4):
                            nc.tensor.matmul(
                                tp[:, s * 128:(s + 1) * 128],
                                ctn[:, s * 128:(s + 1) * 128],
                                identB[:], is_transpose=True,
                                start=(s == 0), stop=(s == 3))
                        nc.vector.tensor_copy(
                            ctxT_sb[hp][:, j * 512:(j + 1) * 512], tp[:])

                    # out-proj filler first (~1.4us of ready PE work), then the
                    # transpose tail so its DVE ctn dependency is surely met
                    if j >= 1:
                        queue_out_proj(j - 1, hp)
                    workq.append(norm_tail)

            def kernel_body(first):
                for n in range(NCHUNK):
                    if n == 0:
                        # chunk 0 projections run dense (nothing to overlap)
                        queue_qk_proj(0, qT, wq_sb, qhT_sb, qin, "q", bq_sb)
                        flush_all()
                        if first:
                            late_const_loads()
                            for kp in range(KT // 2):
                                nc.sync.dma_start(
                                    out=wkw.rearrange("p (k c) -> p k c", c=DOUT)[:, 2 * kp:2 * kp + 2, :],
                                    in_=wk.rearrange("(k p) c -> p k c", p=128)[:, 2 * kp:2 * kp + 2, :])
                        queue_qk_proj(0, kT, wk_sb, khT_sb, kin, "k", None)
                        flush_all()
                        if first:
                            for kp in range(KT // 2):
                                nc.sync.dma_start(
                                    out=wvw.rearrange("p (k c) -> p k c", c=DOUT)[:, 2 * kp:2 * kp + 2, :],
                                    in_=wv.rearrange("(k p) c -> p k c", p=128)[:, 2 * kp:2 * kp + 2, :])
                            nc.sync.dma_start(out=bvbc_sb[:], in_=bvbc[:])
                        queue_v_proj(0, first)
                        flush_all()
                        if first:
                            for kp in range(2):
                                nc.sync.dma_start(
                                    out=wow.rearrange("p (k c) -> p k c", c=D)[:, 2 * kp:2 * kp + 2, :],
                                    in_=wo.rearrange("(k p) c -> p k c", p=128)[:, 2 * kp:2 * kp + 2, :])
                            nc.sync.dma_start(out=bobc_sb[:], in_=bobc[:])
                    # queue next chunk's projections; they drain inside this
                    # chunk's attention i-loops (PE gap filler)
                    if n + 1 < NCHUNK:
                        queue_qk_proj(n + 1, qT, wq_sb, qhT_sb, qin, "q", bq_sb)
                        queue_qk_proj(n + 1, kT, wk_sb, khT_sb, kin, "k", None)
                        queue_v_proj(n + 1, first)
                    attention_chunk(n)
                    flush_all()
                # final out-proj (chunk 3 pieces) — runs dense after the last
                # transpose tail flushed above
                for mt2 in range(3):
                    queue_out_proj(NCHUNK - 1, mt2)
                flush_all()
                # last piece: per-half DMAs so the first half's writeback
                # overlaps the second half's bias add
                rowL = (NCHUNK - 1) * 512 + 3 * 128
                obL = outsb.tile([128, 1024], F32, name="ob", tag="ob")
                for n2 in range(2):
                    ps = mmps.tile([128, 512], F32, name="mmtile", tag="mmtile")
                    for ktile in range(4):
                        nc.tensor.matmul(
                            ps[:],
                            ctxT_sb[ktile][:, rowL:rowL + 128],
                            wo_sb[ktile][:, n2 * 512:(n2 + 1) * 512],
                            start=(ktile == 0), stop=(ktile == 3))
                    nc.vector.tensor_add(
                        obL[:, n2 * 512:(n2 + 1) * 512], ps[:],
                        bobc_sb[:, n2 * 512:(n2 + 1) * 512])
                    nc.sync.dma_start(
                        out=out[rowL:rowL + 128, n2 * 512:(n2 + 1) * 512],
                        in_=obL[:, n2 * 512:(n2 + 1) * 512])

            kernel_body(first=True)
            for _ in range(reps - 1):
                kernel_body(first=False)

    _legalize_waits(nc)
    return nc


def get_program():
    global _PROGRAM
    if _PROGRAM is None:
        _PROGRAM = build_program()
    return _PROGRAM


def make_in_maps(q, k, v, Wq, bq, Wk, bk, Wv, bv, Wo, bo):
    bf = ml_dtypes.bfloat16
    q = np.asarray(q, np.float32); k = np.asarray(k, np.float32)
    v = np.asarray(v, np.float32)
    Wq = np.asarray(Wq, np.float32); Wk = np.asarray(Wk, np.float32)
    Wv = np.asarray(Wv, np.float32); Wo = np.asarray(Wo, np.float32)
    bq = np.asarray(bq, np.float32); bv = np.asarray(bv, np.float32)
    bo = np.asarray(bo, np.float32)

    # causal sliding mask table: B[p, c] = 1.0 iff c >= p + 512
    p = np.arange(128)[:, None]
    c = np.arange(1024)[None, :]
    maskB = (c >= p + 512).astype(bf)

    qTb = [np.ascontiguousarray(q[b].T.astype(bf)) for b in range(B)]
    kTb = [np.ascontiguousarray(k[b].T.astype(bf)) for b in range(B)]
    vTb = [np.ascontiguousarray(v[b].T.astype(bf)) for b in range(B)]

    in_maps = []
    for core in range(8):
        b, hg = core // 2, core % 2
        hs = hg * DOUT
        in_maps.append({
            "qT": qTb[b], "kT": kTb[b], "vT": vTb[b],
            "wq": np.ascontiguousarray(Wq[:, hs:hs + DOUT].astype(bf)),
            "wk": np.ascontiguousarray(Wk[:, hs:hs + DOUT].astype(bf)),
            "wv": np.ascontiguousarray(Wv[:, hs:hs + DOUT].astype(bf)),
            "wo": np.ascontiguousarray(Wo[hs:hs + DOUT, :].astype(bf)),
            "bq_col": np.ascontiguousarray(bq[hs:hs + DOUT].reshape(DOUT, 1)),
            "bvbc": np.ascontiguousarray(
                np.tile(bv[hs:hs + DOUT].reshape(1, DOUT), (128, 1)).astype(bf)),
            "bobc": np.ascontiguousarray(
                np.tile((bo / 2.0).reshape(1, D), (128, 1)).astype(np.float32)),
            "maskB": maskB,
        })
    return in_maps


def assemble_output(results):
    out = np.empty((B, L, D), np.float32)
    for b in range(B):
        out[b] = results[2 * b]["out"] + results[2 * b + 1]["out"]
    return out


def kernel(q, k, v, attn_mask, Wq, bq, Wk, bk, Wv, bv, Wo, bo):
    from concourse.bass_utils import run_bass_kernel_spmd

    nc = get_program()
    in_maps = make_in_maps(q, k, v, Wq, bq, Wk, bk, Wv, bv, Wo, bo)
    last_err = None
    for _ in range(3):  # retry transient device errors (NRT_EXEC_UNIT_...)
        try:
            res = run_bass_kernel_spmd(nc, in_maps, list(range(8)), trace=False)
            return assemble_output(res.results)
        except Exception as e:  # noqa: BLE001
            last_err = e
    raise last_err


# revision 4
# speedup vs baseline: 2.3596x; 2.0954x over previous
"""Multi-head attention (B=4, L=2048, D=1024, H=16, causal) on 8 trn2 cores.

Sharding: core c handles batch b=c//2 and head-group hg=c%2 (8 heads = 512 of
the 1024 projection dims).  Each core computes Q/K/V projections for its
(batch, head-group), causal attention for its 8 heads, and a partial output
projection; the two cores sharing a batch produce additive partials the host
sums.

vs kernel2 (flipped-PV): the attention i-loop is ACT-bound (exp) while the
projection phases are PE-bound, so kernel3 dissolves the phase boundary: all
non-attention PE work (next chunk's Q/K/V projection matmuls, prior chunk's
output-projection pieces, the deferred ctx transposes) flows through a global
FIFO work queue drained a few units after each P@V group inside the attention
i-loop — PE fills its exp-wait gaps with projection work.  Scores are emitted
one i ahead (sp double buffer) so exp(i) overlaps scores(i+1) instead of
stalling PE.  Projection drains move from ACT to DVE (ACT must stay exp-only
once projections overlap attention), and the ctx transposes run bf16 (half
the PE rows of f32) into a bitcast view of an mmps PSUM tile.

Inputs arrive fp32; the host pre-transposes/casts q/k/v to bf16 [D, L] per
core.  All matmuls run bf16 with fp32 PSUM accumulation.
"""

import os as _os
from collections import deque
import numpy as np
import ml_dtypes

import concourse.bass as bass
import concourse.mybir as mybir
from concourse.tile import TileContext
from concourse.masks import make_identity

BF16 = mybir.dt.bfloat16
F32 = mybir.dt.float32

B, L, D, H, DK = 4, 2048, 1024, 16, 64
HPC = 8            # heads per core
DOUT = 512         # projection dims per core
NCHUNK = 4         # lq chunks of 512
KT = 8             # k-tiles over D
SCALE = 1.0 / np.sqrt(DK)

_PROGRAM = None


def _legalize_waits(nc):
    """This walrus build rejects >1 semaphore wait per instruction; split
    extras onto single-wait no-op carriers inserted just before, same engine."""
    for fn in nc.m.functions:
        for blk in fn.blocks:
            insts = blk.instructions
            i = 0
            while i < len(insts):
                inst = insts[i]
                si = inst.sync_info
                waits = list(si.on_wait) if (si and si.on_wait) else []
                if len(waits) > 1:
                    si.on_wait = waits[-1:]
                    carriers = [
                        mybir.InstNoOp(
                            name=nc.get_next_instruction_name(),
                            engine=inst.engine,
                            ins=[],
                            outs=[],
                            sync_info=mybir.SyncInfo(on_wait=[w], on_update=[]),
                        )
                        for w in waits[:-1]
                    ]
                    insts[i:i] = carriers
                    i += len(carriers)
                i += 1


def build_program(reps=1, num_devices=8):
    """reps>1 repeats the whole kernel body inside one NEFF — used only by
    the benchmark to amortize the per-dispatch cost."""
    nc = bass.Bass("TRN2", target_bir_lowering=False, debug=False,
                   num_devices=num_devices)

    qT = nc.declare_dram_parameter("qT", [D, L], BF16, isOutput=False)
    kT = nc.declare_dram_parameter("kT", [D, L], BF16, isOutput=False)
    vT = nc.declare_dram_parameter("vT", [D, L], BF16, isOutput=False)
    wq = nc.declare_dram_parameter("wq", [D, DOUT], BF16, isOutput=False)
    wk = nc.declare_dram_parameter("wk", [D, DOUT], BF16, isOutput=False)
    wv = nc.declare_dram_parameter("wv", [D, DOUT], BF16, isOutput=False)
    wo = nc.declare_dram_parameter("wo", [DOUT, D], BF16, isOutput=False)
    bq_col = nc.declare_dram_parameter("bq_col", [DOUT, 1], F32, isOutput=False)
    bvbc = nc.declare_dram_parameter("bvbc", [128, DOUT], BF16, isOutput=False)
    bobc = nc.declare_dram_parameter("bobc", [128, D], F32, isOutput=False)
    maskB = nc.declare_dram_parameter("maskB", [128, 1024], BF16, isOutput=False)
    out = nc.declare_dram_parameter("out", [L, D], F32, isOutput=True)

    with TileContext(nc) as tc:
        with (
            tc.tile_pool(name="wpool", bufs=1) as wpool,
            tc.tile_pool(name="vtpool", bufs=1) as vtpool,
            tc.tile_pool(name="big", bufs=1) as big,
            tc.tile_pool(name="qin", bufs=int(_os.environ.get("K_QIN", "6"))) as qin,
            tc.tile_pool(name="kin", bufs=int(_os.environ.get("K_KIN", "6"))) as kin,
            tc.tile_pool(name="ppool", bufs=int(_os.environ.get("K_PPOOL", "4"))) as ppool,
            tc.tile_pool(name="cscp", bufs=3) as cscp,
            tc.tile_pool(name="rcp", bufs=3) as rcp,
            tc.tile_pool(name="outsb", bufs=int(_os.environ.get("K_OUTSB", "4"))) as outsb,
            tc.tile_pool(name="mmps", bufs=2, space="PSUM") as mmps,
            tc.tile_pool(name="sps", bufs=2, space="PSUM") as spsp,
            tc.tile_pool(name="ctxps", bufs=1, space="PSUM") as ctxps,
        ):
            # ---- constants / weights ----
            wq_sb = [wpool.tile([128, DOUT], BF16, name=f"wq{k}", tag=f"wq{k}") for k in range(KT)]
            wkw = wpool.tile([128, KT * DOUT], BF16, name="wkw", tag="wkw")
            wvw = wpool.tile([128, KT * DOUT], BF16, name="wvw", tag="wvw")
            wow = wpool.tile([128, 4 * D], BF16, name="wow", tag="wow")
            wk_sb = [wkw[:, k * DOUT:(k + 1) * DOUT] for k in range(KT)]
            wv_sb = [wvw[:, k * DOUT:(k + 1) * DOUT] for k in range(KT)]
            wo_sb = [wow[:, k * D:(k + 1) * D] for k in range(4)]
            bvbc_sb = wpool.tile([128, DOUT], BF16, name="bvbc_sb", tag="bvbc_sb")
            bobc_sb = wpool.tile([128, D], F32, name="bobc_sb", tag="bobc_sb")
            for k in range(2):
                nc.sync.dma_start(out=wq_sb[k][:], in_=wq[k * 128:(k + 1) * 128, :])
            mask_sb = wpool.tile([128, 1024], BF16, name="mask_sb", tag="mask_sb")
            bq4 = wpool.tile([128, 4], F32, name="bq4", tag="bq4")
            nc.sync.dma_start(
                out=bq4[:], in_=bq_col.rearrange("(m p) one -> p (m one)", p=128))
            bq_sb = [bq4[:, m:m + 1] for m in range(4)]
            # bf16 identity for the PE ctx transposes
            identB = wpool.tile([128, 128], BF16, name="identB", tag="identB")
            make_identity(nc, identB[:])

            def late_const_loads():
                nc.sync.dma_start(out=mask_sb[:], in_=maskB[:])

            bigvT = vtpool.tile([128, KT * L], BF16, name="bigvT", tag="bigvT")
            vT_sb = [bigvT[:, k * L:(k + 1) * L] for k in range(KT)]

            # ---- persistent activations ----
            qhT_sb = [big.tile([128, L], BF16, name=f"qhT{m}", tag=f"qhT{m}") for m in range(4)]
            khT_sb = [big.tile([128, L], BF16, name=f"khT{m}", tag=f"khT{m}") for m in range(4)]
            ctxT_sb = [big.tile([128, L], BF16, name=f"ctxT{m}", tag=f"ctxT{m}") for m in range(4)]
            vh_sb = [big.tile([128, HPC * 65], BF16, name=f"vh{t}", tag=f"vh{t}") for t in range(16)]
            for t in range(16):
                nc.vector.memset(
                    vh_sb[t].rearrange("p (h c) -> p h c", c=65)[:, :, 64:65], 1.0)

            # ---- global PE filler work queue ----
            # closures that emit one small PE unit (plus its drains); drained
            # K_DRAIN at a time after each P@V group inside the attention
            # i-loop, so PE fills its exp-wait gaps with projection work
            workq = deque()
            DRAIN = int(_os.environ.get("K_DRAIN", "2"))

            def drain(k=DRAIN):
                for _ in range(min(k, len(workq))):
                    workq.popleft()()

            def flush_all():
                while workq:
                    workq.popleft()()

            def queue_qk_proj(n, src, w_sb, dst, pool, pfx, bias):
                state = {}

                def dmas():
                    tiles = []
                    s3 = src.rearrange("(k p) c -> p k c", p=128)
                    for kp in range(KT // 2):
                        t = pool.tile([128, 1024], BF16, name=f"{pfx}in", tag=f"{pfx}in")
                        nc.sync.dma_start(
                            out=t.rearrange("p (k c) -> p k c", c=512),
                            in_=s3[:, 2 * kp:2 * kp + 2, n * 512:(n + 1) * 512])
                        tiles.append(t[:, 0:512])
                        tiles.append(t[:, 512:1024])
                    state["tiles"] = tiles
                workq.append(dmas)

                for m in range(4):
                    def mk(m=m, k=0):
                        ps = mmps.tile([128, 512], F32, name="mmtile", tag="mmtile")
                        state[f"ps{m}"] = ps
                        nc.tensor.matmul(
                            ps[:], w_sb[0][:, m * 128:(m + 1) * 128],
                            state["tiles"][0][:], start=True, stop=False)
                    workq.append(mk)
                    for k in range(1, KT, 2):
                        def mk2(m=m, k=k):
                            ps = state[f"ps{m}"]
                            for kk in (k, k + 1):
                                if kk < KT:
                                    nc.tensor.matmul(
                                        ps[:], w_sb[kk][:, m * 128:(m + 1) * 128],
                                        state["tiles"][kk][:],
                                        start=False, stop=(kk == KT - 1))
                        workq.append(mk2)

                    def mdrain(m=m):
                        ps = state[f"ps{m}"]
                        dsts = dst[m][:, n * 512:(n + 1) * 512]
                        # DVE, not ACT: these drains now overlap the previous
                        # chunk's attention and ACT must stay exp-only
                        if bias is not None:
                            nc.vector.tensor_scalar_add(dsts, ps[:], bias[m][:])
                        else:
                            nc.vector.tensor_copy(dsts, ps[:])
                    workq.append(mdrain)

            def queue_v_proj(n, first):
                def vdma():
                    if not first:
                        return
                    bv3 = bigvT.rearrange("p (k c) -> p k c", c=L)
                    vs3 = vT.rearrange("(k p) c -> p k c", p=128)
                    for kp in range(KT // 2):
                        nc.sync.dma_start(
                            out=bv3[:, 2 * kp:2 * kp + 2, n * 512:(n + 1) * 512],
                            in_=vs3[:, 2 * kp:2 * kp + 2, n * 512:(n + 1) * 512])
                workq.append(vdma)
                state = {}
                for mt in range(4 * n, 4 * n + 4):
                    def vk(mt=mt, k=0):
                        ps = mmps.tile([128, 512], F32, name="mmtile", tag="mmtile")
                        state[f"ps{mt}"] = ps
                        nc.tensor.matmul(
                            ps[:], vT_sb[0][:, mt * 128:(mt + 1) * 128], wv_sb[0][:],
                            start=True, stop=False)
                    workq.append(vk)
                    for k in range(1, KT, 2):
                        def vk2(mt=mt, k=k):
                            ps = state[f"ps{mt}"]
                            for kk in (k, k + 1):
                                if kk < KT:
                                    nc.tensor.matmul(
                                        ps[:], vT_sb[kk][:, mt * 128:(mt + 1) * 128],
                                        wv_sb[kk][:], start=False,
                                        stop=(kk == KT - 1))
                        workq.append(vk2)

                    def vdrain(mt=mt):
                        ps = state[f"ps{mt}"]
                        dst3 = vh_sb[mt].rearrange("p (h c) -> p h c", c=65)
                        nc.vector.tensor_add(
                            dst3[:, :, 0:64], ps.rearrange("p (h c) -> p h c", c=64),
                            bvbc_sb.rearrange("p (h c) -> p h c", c=64))
                    workq.append(vdrain)

            def queue_out_proj(jj, mt2):
                state = {}

                def half(n2):
                    ps = mmps.tile([128, 512], F32, name="mmtile", tag="mmtile")
                    row0 = jj * 512 + mt2 * 128
                    for ktile in range(4):
                        nc.tensor.matmul(
                            ps[:],
                            ctxT_sb[ktile][:, row0:row0 + 128],
                            wo_sb[ktile][:, n2 * 512:(n2 + 1) * 512],
                            start=(ktile == 0), stop=(ktile == 3))
                    if n2 == 0:
                        state["ob"] = outsb.tile([128, 1024], F32, name="ob", tag="ob")
                    ob = state["ob"]
                    nc.vector.tensor_add(
                        ob[:, n2 * 512:(n2 + 1) * 512], ps[:],
                        bobc_sb[:, n2 * 512:(n2 + 1) * 512])
                    if n2 == 1:
                        nc.sync.dma_start(
                            out=out[row0:row0 + 128, :], in_=ob[:])
                workq.append(lambda: half(0))
                workq.append(lambda: half(1))

            def attention_chunk(j):
                ilast = 4 * (j + 1) - 1

                def scores(i):
                    d = max(0, (i - 4 * j) * 128)
                    sp = spsp.tile([128, 1024], F32, name="sps", tag="sps")
                    for h2, po in enumerate((0, 64)):
                        nc.tensor.matmul(
                            sp[:, h2 * 512 + d:(h2 + 1) * 512],
                            khT_sb[hp][po:po + 64, i * 128:(i + 1) * 128],
                            qhT_sb[hp][po:po + 64, j * 512 + d:(j + 1) * 512],
                            start=True, stop=True, tile_position=(po, 0))
                    return sp

                for hp in range(4):
                    ctxs = (
                        ctxps.tile([128, 4 * 65], F32, name="ctxA", tag="ctxA"),
                        ctxps.tile([128, 4 * 65], F32, name="ctxB", tag="ctxB"),
                    )
                    sp = scores(0)
                    for i in range(ilast + 1):
                        d = max(0, (i - 4 * j) * 128)
                        straddle = i >= 4 * j
                        pt = ppool.tile([128, 1024], BF16, name="pt", tag="pt")
                        if d == 0:
                            nc.scalar.activation(
                                out=pt[:], in_=sp[:],
                                func=mybir.ActivationFunctionType.Exp, scale=float(SCALE))
                        else:
                            sp3 = sp.rearrange("p (g c) -> p g c", g=2)
                            pt3 = pt.rearrange("p (g c) -> p g c", g=2)
                            nc.scalar.activation(
                                out=pt3[:, :, d:512], in_=sp3[:, :, d:512],
                                func=mybir.ActivationFunctionType.Exp, scale=float(SCALE))
                        if i < ilast:
                            sp = scores(i + 1)  # one ahead: overlaps exp(i)
                        if straddle:
                            nc.gpsimd.tensor_mul(
                                pt[:, d:d + 128], pt[:, d:d + 128],
                                mask_sb[:, 512:640])
                            nc.gpsimd.tensor_mul(
                                pt[:, 512 + d:512 + d + 128],
                                pt[:, 512 + d:512 + d + 128],
                                mask_sb[:, 512:640])
                        smin = max(0, i - 4 * j)
                        for h2 in range(2):
                            h = hp * 2 + h2
                            for s in range(smin, 4):
                                # one PSUM start per ctx bank: start clears
                                # has_written for the whole bank; later
                                # start=False writes overwrite where clear,
                                # accumulate where set
                                nc.tensor.matmul(
                                    ctxs[h2][:, 65 * s:65 * s + 65],
                                    pt[:, h2 * 512 + s * 128:h2 * 512 + (s + 1) * 128],
                                    vh_sb[i][:, h * 65:(h + 1) * 65],
                                    start=(i == 0 and s == 0),
                                    stop=(i == ilast and s == 3))
                        drain()
                    # inline DVE tail: per-partition denominators -> recip ->
                    # normalize+drain ctx into SBUF bf16 (for the transpose)
                    rc = rcp.tile([128, 8], F32, name="rc", tag="rc")
                    rc3 = rc.rearrange("p (s one) -> p s one", one=1)
                    ctn = cscp.tile([128, 512], BF16, name="ctn", tag="ctn")
                    for h2 in range(2):
                        ctx3 = ctxs[h2].rearrange("p (s c) -> p s c", c=65)
                        nc.vector.reciprocal(
                            out=rc3[:, 4 * h2:4 * h2 + 4, :],
                            in_=ctx3[:, :, 64:65])
                        for s in range(4):
                            nc.vector.tensor_scalar_mul(
                                ctn[:, s * 128 + h2 * 64:s * 128 + h2 * 64 + 64],
                                ctxs[h2][:, 65 * s:65 * s + 64],
                                rc[:, 4 * h2 + s:4 * h2 + s + 1])

                    def norm_tail(hp=hp, j=j, ctn=ctn):
                        # bf16 transposes into a bitcast view of an mmps tile
                        tpf = mmps.tile([128, 512], F32, name="mmtile", tag="mmtile")
                        tp = tpf.bitcast(BF16)[:, 0:512]
                        for s in range(4):
                            nc.tensor.matmul(
                                tp[:, s * 128:(s + 1) * 128],
                                ctn[:, s * 128:(s + 1) * 128],
                                identB[:], is_transpose=True,
                                start=(s == 0), stop=(s == 3))
                        nc.vector.tensor_copy(
                            ctxT_sb[hp][:, j * 512:(j + 1) * 512], tp[:])

                    # out-proj filler first (~1.4us of ready PE work), then the
                    # transpose tail so its DVE ctn dependency is surely met.
                    # chunk 3's attention is the longest and has no next-chunk
                    # projection filler, so it absorbs extra out-proj pieces.
                    if j == 1:
                        queue_out_proj(0, hp)
                    elif j == 2 and hp < 2:
                        queue_out_proj(1, hp)
                    elif j == 3:
                        if hp < 2:
                            queue_out_proj(1, 2 + hp)
                        queue_out_proj(2, hp)
                    workq.append(norm_tail)

            def kernel_body(first):
                for n in range(NCHUNK):
                    if n == 0:
                        # chunk 0 projections run dense (nothing to overlap);
                        # wq2..7 DMAs issue after the first q-tile DMAs so the
                        # first matmuls' dependencies clear the SP queue first
                        queue_qk_proj(0, qT, wq_sb, qhT_sb, qin, "q", bq_sb)
                        workq.popleft()()  # q-tile DMAs
                        for k in range(2, KT):
                            nc.sync.dma_start(
                                out=wq_sb[k][:], in_=wq[k * 128:(k + 1) * 128, :])
                        flush_all()
                        if first:
                            late_const_loads()
                            for kp in range(KT // 2):
                                nc.sync.dma_start(
                                    out=wkw.rearrange("p (k c) -> p k c", c=DOUT)[:, 2 * kp:2 * kp + 2, :],
                                    in_=wk.rearrange("(k p) c -> p k c", p=128)[:, 2 * kp:2 * kp + 2, :])
                        queue_qk_proj(0, kT, wk_sb, khT_sb, kin, "k", None)
                        flush_all()
                        if first:
                            for kp in range(KT // 2):
                                nc.sync.dma_start(
                                    out=wvw.rearrange("p (k c) -> p k c", c=DOUT)[:, 2 * kp:2 * kp + 2, :],
                                    in_=wv.rearrange("(k p) c -> p k c", p=128)[:, 2 * kp:2 * kp + 2, :])
                            nc.sync.dma_start(out=bvbc_sb[:], in_=bvbc[:])
                        queue_v_proj(0, first)
                        flush_all()
                        if first:
                            for kp in range(2):
                                nc.sync.dma_start(
                                    out=wow.rearrange("p (k c) -> p k c", c=D)[:, 2 * kp:2 * kp + 2, :],
                                    in_=wo.rearrange("(k p) c -> p k c", p=128)[:, 2 * kp:2 * kp + 2, :])
                            nc.sync.dma_start(out=bobc_sb[:], in_=bobc[:])
                    # queue next chunk's projections; they drain inside this
                    # chunk's attention i-loops (PE gap filler)
                    if n + 1 < NCHUNK:
                        queue_qk_proj(n + 1, qT, wq_sb, qhT_sb, qin, "q", bq_sb)
                        queue_qk_proj(n + 1, kT, wk_sb, khT_sb, kin, "k", None)
                        queue_v_proj(n + 1, first)
                    if n < NCHUNK - 1:
                        attention_chunk(n)
                        flush_all()
                        continue
                    attention_chunk(n)
                    # stage the first final piece's ktile0-2 matmuls (in sps
                    # halves — mmps is needed by the pending transpose tail)
                    # so they overlap the last head-pair's transpose chain
                    row0 = (NCHUNK - 1) * 512
                    spf = spsp.tile([128, 1024], F32, name="sps", tag="sps")
                    ps0 = [spf[:, 0:512], spf[:, 512:1024]]
                    for n2 in range(2):
                        for ktile in range(3):
                            nc.tensor.matmul(
                                ps0[n2],
                                ctxT_sb[ktile][:, row0:row0 + 128],
                                wo_sb[ktile][:, n2 * 512:(n2 + 1) * 512],
                                start=(ktile == 0), stop=False)
                    flush_all()
                    ob0 = outsb.tile([128, 1024], F32, name="ob", tag="ob")
                    for n2 in range(2):
                        nc.tensor.matmul(
                            ps0[n2],
                            ctxT_sb[3][:, row0:row0 + 128],
                            wo_sb[3][:, n2 * 512:(n2 + 1) * 512],
                            start=False, stop=True)
                        nc.vector.tensor_add(
                            ob0[:, n2 * 512:(n2 + 1) * 512], ps0[n2],
                            bobc_sb[:, n2 * 512:(n2 + 1) * 512])
                    nc.sync.dma_start(out=out[row0:row0 + 128, :], in_=ob0[:])
                for mt2 in (1, 2):
                    queue_out_proj(NCHUNK - 1, mt2)
                flush_all()
                # last piece: per-half DMAs so the first half's writeback
                # overlaps the second half's bias add (quarter-splitting is
                # WORSE: each dma_start costs ~800ns of SP descriptor time)
                rowL = (NCHUNK - 1) * 512 + 3 * 128
                obL = outsb.tile([128, 1024], F32, name="ob", tag="ob")
                for n2 in range(2):
                    ps = mmps.tile([128, 512], F32, name="mmtile", tag="mmtile")
                    for ktile in range(4):
                        nc.tensor.matmul(
                            ps[:],
                            ctxT_sb[ktile][:, rowL:rowL + 128],
                            wo_sb[ktile][:, n2 * 512:(n2 + 1) * 512],
                            start=(ktile == 0), stop=(ktile == 3))
                    nc.vector.tensor_add(
                        obL[:, n2 * 512:(n2 + 1) * 512], ps[:],
                        bobc_sb[:, n2 * 512:(n2 + 1) * 512])
                    nc.sync.dma_start(
                        out=out[rowL:rowL + 128, n2 * 512:(n2 + 1) * 512],
                        in_=obL[:, n2 * 512:(n2 + 1) * 512])

            kernel_body(first=True)
            for _ in range(reps - 1):
                kernel_body(first=False)

    _legalize_waits(nc)
    return nc


def get_program():
    global _PROGRAM
    if _PROGRAM is None:
        _PROGRAM = build_program()
    return _PROGRAM


def make_in_maps(q, k, v, Wq, bq, Wk, bk, Wv, bv, Wo, bo):
    bf = ml_dtypes.bfloat16
    q = np.asarray(q, np.float32); k = np.asarray(k, np.float32)
    v = np.asarray(v, np.float32)
    Wq = np.asarray(Wq, np.float32); Wk = np.asarray(Wk, np.float32)
    Wv = np.asarray(Wv, np.float32); Wo = np.asarray(Wo, np.float32)
    bq = np.asarray(bq, np.float32); bv = np.asarray(bv, np.float32)
    bo = np.asarray(bo, np.float32)

    # causal sliding mask table: B[p, c] = 1.0 iff c >= p + 512
    p = np.arange(128)[:, None]
    c = np.arange(1024)[None, :]
    maskB = (c >= p + 512).astype(bf)

    qTb = [np.ascontiguousarray(q[b].T.astype(bf)) for b in range(B)]
    kTb = [np.ascontiguousarray(k[b].T.astype(bf)) for b in range(B)]
    vTb = [np.ascontiguousarray(v[b].T.astype(bf)) for b in range(B)]

    in_maps = []
    for core in range(8):
        b, hg = core // 2, core % 2
        hs = hg * DOUT
        in_maps.append({
            "qT": qTb[b], "kT": kTb[b], "vT": vTb[b],
            "wq": np.ascontiguousarray(Wq[:, hs:hs + DOUT].astype(bf)),
            "wk": np.ascontiguousarray(Wk[:, hs:hs + DOUT].astype(bf)),
            "wv": np.ascontiguousarray(Wv[:, hs:hs + DOUT].astype(bf)),
            "wo": np.ascontiguousarray(Wo[hs:hs + DOUT, :].astype(bf)),
            "bq_col": np.ascontiguousarray(bq[hs:hs + DOUT].reshape(DOUT, 1)),
            "bvbc": np.ascontiguousarray(
                np.tile(bv[hs:hs + DOUT].reshape(1, DOUT), (128, 1)).astype(bf)),
            "bobc": np.ascontiguousarray(
                np.tile((bo / 2.0).reshape(1, D), (128, 1)).astype(np.float32)),
            "maskB": maskB,
        })
    return in_maps


def assemble_output(results):
    out = np.empty((B, L, D), np.float32)
    for b in range(B):
        out[b] = results[2 * b]["out"] + results[2 * b + 1]["out"]
    return out


def kernel(q, k, v, attn_mask, Wq, bq, Wk, bk, Wv, bv, Wo, bo):
    from concourse.bass_utils import run_bass_kernel_spmd

    nc = get_program()
    in_maps = make_in_maps(q, k, v, Wq, bq, Wk, bk, Wv, bv, Wo, bo)
    last_err = None
    for _ in range(3):  # retry transient device errors (NRT_EXEC_UNIT_...)
        try:
            res = run_bass_kernel_spmd(nc, in_maps, list(range(8)), trace=False)
            return assemble_output(res.results)
        except Exception as e:  # noqa: BLE001
            last_err = e
    raise last_err
